# revision 15
# baseline (speedup 1.0000x reference)
"""Trainium2 Bass kernel for nn_DownsampleMultiscale (embedding_lookup).

reference:
    xf = x.reshape(2, H*W)                      # x: (2, 256, 256) f32
    y_lo = (xf @ X_lo.T).reshape(2, 32, 32)     # X_lo: (1024, 65536) one-hot rows
    y_hi = (xf @ X_hi.T).reshape(2, 64, 64)     # X_hi: (4096, 65536) one-hot rows

Each row of X_lo / X_hi is one-hot, so each output pixel is a single lookup
into the image: y[c, i] = v_i * x[c, j_i], where j_i is the row's nonzero
column and v_i its value (1.0 here). The host compresses the one-hot
matrices to (j, v) pairs; the device does the lookups from the full image.

Sharding (per the hint): data-parallel over output pixels — each of the 8
NeuronCores produces 1/8 of the downsampled grids (8 of 64 y_hi rows + 4 of
32 y_lo rows). x is replicated.

Fast path (the formula matrices are separable: j = ry[u]*W + cx[v]):
  - per core, ONE indirect DMA gathers its 24 needed image rows (2 ch x 12
    rows, 1KB descriptors) using a small runtime row-index input,
  - the column subsampling (cx) is compile-time: 12 strided tensor-copies
    (split across the vector + gpsimd engines) compact the selected
    columns, one plain DMA writes the core's output slice.
Generic path (any X with single-nonzero rows; also scaled/zero rows):
  - per-element gather: 5 indirect DMAs of 128 single-pixel descriptors
    (both channels per descriptor), optional scale multiply.
Matrices with multi-nonzero rows decompose into L one-hot layers -> L
generic-path launches, summed on the host (L == 1 in practice).
"""

import numpy as np

import concourse.bass as bass
import concourse.mybir as mybir
from concourse.bass_utils import run_bass_kernel_spmd

H, W = 256, 256
HW = H * W                               # 65536
SZ_LO, SZ_HI = 32, 64
N_LO = SZ_LO * SZ_LO                     # 1024
N_HI = SZ_HI * SZ_HI                     # 4096
N_OUT = N_LO + N_HI                      # 5120
N_CORES = 8
PER_CORE = N_OUT // N_CORES              # 640
P = 128
CHUNK = PER_CORE // P                    # 5

HI_PER_CORE = SZ_HI // N_CORES           # 8 y_hi rows per core
LO_PER_CORE = SZ_LO // N_CORES           # 4 y_lo rows per core
N_HI_ROWS = 2 * HI_PER_CORE              # 16 gathered (ch, row) pairs
N_LO_ROWS = 2 * LO_PER_CORE              # 8
LO_BASE = 32                             # lo partition base (32-aligned)
NROWS = LO_BASE + N_LO_ROWS              # 40 gather partitions

F32 = mybir.dt.float32
I32 = mybir.dt.int32

_PROGRAM_CACHE: dict = {}


def _new_bass():
    """Bass() without the init all-engine barrier / NRT pseudo-barrier /
    const-AP memsets / monotonic sems — none of which this kernel needs."""
    orig_pb = bass.Bass._nrt_pseudo_barrier
    orig_aeb = bass.Bass.all_engine_barrier
    bass.Bass._nrt_pseudo_barrier = lambda self: None
    bass.Bass.all_engine_barrier = lambda self, *, sem_only=False: None
    try:
        nc = bass.Bass(monotonic_sem_count=0)
    finally:
        bass.Bass._nrt_pseudo_barrier = orig_pb
        bass.Bass.all_engine_barrier = orig_aeb
    bb = nc.main_func.blocks[0]
    for inst in [i for i in bb.instructions if type(i).__name__ == "InstMemset"]:
        bb.instructions.remove(inst)
    return nc


def _segments(cx):
    """Split a column-index list into maximal dominant-stride segments.

    Returns [(dst_pos, src_start, stride, count)] covering cx in order."""
    from collections import Counter

    cx = list(map(int, cx))
    if len(cx) == 1:
        return [(0, cx[0], 1, 1)]
    d = [b - a for a, b in zip(cx, cx[1:])]
    s_dom = Counter(d).most_common(1)[0][0]
    if s_dom < 1:
        s_dom = 1
    segs = []
    start = 0
    for t in range(1, len(cx)):
        if d[t - 1] != s_dom:
            segs.append((start, cx[start], s_dom, t - start))
            start = t
    segs.append((start, cx[start], s_dom, len(cx) - start))
    return segs


def _pair_jobs(p0, p1, segs):
    """Pair segments with identical (stride, count) into merged copy jobs.

    Job: (p0, p1, dpos, src0, stride, cnt, dgap, sgap); dgap == 0 -> single
    segment, dgap > 0 -> two segments (second at dpos+dgap / src0+sgap)."""
    from collections import defaultdict

    groups = defaultdict(list)
    for d, s, st, c in segs:
        groups[(st, c)].append((d, s))
    jobs = []
    for (st, c), g in groups.items():
        g.sort()
        while g:
            d1, s1 = g.pop(0)
            if g and g[0][0] > d1 and g[0][1] > s1:
                d2, s2 = g.pop(0)
                jobs.append((p0, p1, d1, s1, st, c, d2 - d1, s2 - s1))
            else:
                jobs.append((p0, p1, d1, s1, st, c, 0, 0))
    jobs.sort(key=lambda j: -(j[5] * (2 if j[6] else 1)))
    return jobs


def _build_separable(cx_hi, cx_lo):
    """Row-gather + compile-time column-select program (shared by all cores)."""
    nc = _new_bass()
    xt = nc.dram_tensor("xt", [2 * H, W], F32, kind="ExternalInput")
    ridx = nc.dram_tensor("ridx", [NROWS, 1], I32, kind="ExternalInput")
    out = nc.dram_tensor("out", [NROWS, SZ_HI], F32, kind="ExternalOutput")
    with (
        nc.sbuf_tensor([NROWS, 1], I32) as ridx_t,
        nc.sbuf_tensor([NROWS, W], F32) as rows_t,
        nc.sbuf_tensor([NROWS, SZ_HI], F32) as y_t,
        nc.semaphore() as s_r,
        nc.semaphore() as s_g,
        nc.semaphore() as s_hi,
        nc.semaphore() as s_lo,
        nc.semaphore() as s_out,
    ):
        nc.sync.dma_start(out=ridx_t[:], in_=ridx[:]).then_inc(s_r, 16)
        # zero the lo-region columns the copies don't write (off the critical
        # path: runs in the shadow of the ridx DMA receipt); counted into
        # s_lo so the lo output DMA orders after it
        nc.gpsimd.memset(y_t[LO_BASE:NROWS, SZ_LO:SZ_HI], 0.0).then_inc(s_lo, 1)
        nc.gpsimd.wait_ge(s_r, 16)
        nc.gpsimd.indirect_dma_start(
            out=rows_t[:],
            out_offset=None,
            in_=xt[:],
            in_offset=bass.IndirectOffsetOnAxis(ap=ridx_t[:, 0:1], axis=0),
        ).then_inc(s_g, 16)
        hi_jobs = _pair_jobs(0, N_HI_ROWS, _segments(cx_hi))
        lo_jobs = _pair_jobs(LO_BASE, LO_BASE + N_LO_ROWS, _segments(cx_lo))
        engines = [nc.vector, nc.gpsimd]
        for eng in engines:
            eng.wait_ge(s_g, 16)
        # hi copies first on both engines so the hi output DMA launches while
        # the lo copies are still running
        for jobs, sem in ((hi_jobs, s_hi), (lo_jobs, s_lo)):
            for qi, (p0, p1, dpos, src0, stride, cnt, dgap, sgap) in enumerate(
                jobs
            ):
                eng = engines[qi % len(engines)]
                src = rows_t[p0:p1, src0 : src0 + stride * (cnt - 1) + 1 : stride]
                dst = y_t[p0:p1, dpos : dpos + cnt]
                if dgap:
                    # two segments with identical (stride, count): one copy
                    # with an extra outer AP dim covering both
                    src = bass.AP(
                        src.tensor,
                        src.offset,
                        [list(src.ap[0]), [sgap, 2], list(src.ap[1])],
                    )
                    dst = bass.AP(
                        dst.tensor,
                        dst.offset,
                        [list(dst.ap[0]), [dgap, 2], list(dst.ap[1])],
                    )
                eng.tensor_copy(out=dst, in_=src).then_inc(sem, 1)
        nc.sync.wait_ge(s_hi, len(hi_jobs))
        nc.sync.dma_start(
            out=out[0:N_HI_ROWS, :], in_=y_t[0:N_HI_ROWS, :]
        ).then_inc(s_out, 16)
        nc.sync.wait_ge(s_lo, len(lo_jobs) + 1)
        nc.sync.dma_start(
            out=out[LO_BASE:NROWS, :], in_=y_t[LO_BASE:NROWS, :]
        ).then_inc(s_out, 16)
    return nc


def _build_generic(with_scale):
    """Per-element gather: out (p, 2m+c) <- v * x_T[idx[p, m], c]."""
    nc = _new_bass()
    xt = nc.dram_tensor("xt", [HW, 2], F32, kind="ExternalInput")
    idx = nc.dram_tensor("idx", [P, CHUNK], I32, kind="ExternalInput")
    if with_scale:
        vex = nc.dram_tensor("vex", [P, 2 * CHUNK], F32, kind="ExternalInput")
    out = nc.dram_tensor("out", [P, 2 * CHUNK], F32, kind="ExternalOutput")
    with (
        nc.sbuf_tensor([P, CHUNK], I32) as idx_t,
        nc.sbuf_tensor([P, 2 * CHUNK], F32) as g_t,
        nc.sbuf_tensor([P, 2 * CHUNK], F32) as v_t,
        nc.sbuf_tensor([P, 2 * CHUNK], F32) as y_t,
        nc.semaphore() as s_in,
        nc.semaphore() as s_g,
        nc.semaphore() as s_mul,
        nc.semaphore() as s_out,
    ):
        nc.sync.dma_start(out=idx_t[:], in_=idx[:]).then_inc(s_in, 16)
        if with_scale:
            nc.sync.dma_start(out=v_t[:], in_=vex[:]).then_inc(s_in, 16)
        nc.gpsimd.wait_ge(s_in, 32 if with_scale else 16)
        for m in range(CHUNK):
            nc.gpsimd.indirect_dma_start(
                out=g_t[:, 2 * m : 2 * m + 2],
                out_offset=None,
                in_=xt[:],
                in_offset=bass.IndirectOffsetOnAxis(ap=idx_t[:, m : m + 1], axis=0),
            ).then_inc(s_g, 16)
        if with_scale:
            nc.vector.wait_ge(s_g, 16 * CHUNK)
            nc.vector.tensor_mul(out=y_t[:], in0=g_t[:], in1=v_t[:]).then_inc(
                s_mul, 1
            )
            nc.sync.wait_ge(s_mul, 1)
            nc.sync.dma_start(out=out[:], in_=y_t[:]).then_inc(s_out, 16)
        else:
            nc.sync.wait_ge(s_g, 16 * CHUNK)
            nc.sync.dma_start(out=out[:], in_=g_t[:]).then_inc(s_out, 16)
    return nc


def _get_program(key, builder):
    if key not in _PROGRAM_CACHE:
        _PROGRAM_CACHE[key] = builder()
    return _PROGRAM_CACHE[key]


def _run_spmd(nc, in_maps, attempts=3):
    """run_bass_kernel_spmd with retries for transient device errors."""
    for att in range(attempts):
        try:
            return run_bass_kernel_spmd(
                nc, in_maps, core_ids=list(range(N_CORES))
            ).results
        except Exception:
            if att == attempts - 1:
                raise
            import time

            time.sleep(2.0 * (att + 1))


def _extract_single_nonzero(X):
    """(col, val) per row for a matrix with at most one nonzero per row.

    Returns None if some row has more than one nonzero."""
    rows, cols = X.shape
    pos = np.flatnonzero(X)
    r = pos // cols
    if pos.size and np.any(r[1:] == r[:-1]):
        return None  # some row has several nonzeros
    j = np.zeros(rows, np.int32)
    v = np.zeros(rows, np.float32)
    j[r] = (pos % cols).astype(np.int32)
    v[r] = X.ravel()[pos].astype(np.float32)
    return j, v


def _sparse_layers(X):
    """Decompose X into layers of (j, v) with one nonzero per row each."""
    rows, cols = X.shape
    pos = np.flatnonzero(X)
    r = pos // cols
    layers = []
    k = 0
    while pos.size:
        first = np.ones(pos.size, bool)
        first[1:] = r[1:] != r[:-1]
        sel = pos[first]
        j = np.zeros(rows, np.int32)
        v = np.zeros(rows, np.float32)
        rr = sel // cols
        j[rr] = (sel % cols).astype(np.int32)
        v[rr] = X.ravel()[sel].astype(np.float32)
        layers.append((j, v))
        pos, r = pos[~first], r[~first]
        k += 1
        if k > 64:  # pathological density: bail (cannot happen for one-hot)
            raise ValueError("X matrices are too dense for the gather kernel")
    return layers


def _separable(j, sz):
    Jm = j.reshape(sz, sz)
    ry = Jm[:, 0] // W
    cx = Jm[0, :] % W
    if np.array_equal(Jm, ry[:, None] * W + cx[None, :]):
        return ry.astype(np.int64), cx.astype(np.int64)
    return None


def _run_separable(x, ry_hi, cx_hi, ry_lo, cx_lo):
    nc = _get_program(
        ("sep", tuple(cx_hi.tolist()), tuple(cx_lo.tolist())),
        lambda: _build_separable(cx_hi, cx_lo),
    )
    xt = np.ascontiguousarray(x.reshape(2 * H, W))
    in_maps = []
    for k in range(N_CORES):
        rh = ry_hi[k * HI_PER_CORE : (k + 1) * HI_PER_CORE]
        rl = ry_lo[k * LO_PER_CORE : (k + 1) * LO_PER_CORE]
        ridx = np.zeros(NROWS, np.int64)
        ridx[0:HI_PER_CORE] = rh
        ridx[HI_PER_CORE : 2 * HI_PER_CORE] = H + rh
        ridx[LO_BASE : LO_BASE + LO_PER_CORE] = rl
        ridx[LO_BASE + LO_PER_CORE : LO_BASE + 2 * LO_PER_CORE] = H + rl
        in_maps.append(
            {"xt": xt, "ridx": ridx.astype(np.int32).reshape(NROWS, 1)}
        )
    res = _run_spmd(nc, in_maps)
    y_hi = np.empty((2, SZ_HI, SZ_HI), np.float32)
    y_lo = np.empty((2, SZ_LO, SZ_LO), np.float32)
    for k in range(N_CORES):
        o = np.asarray(res[k]["out"])  # (40, 64)
        for c in range(2):
            y_hi[c, k * HI_PER_CORE : (k + 1) * HI_PER_CORE, :] = o[
                c * HI_PER_CORE : (c + 1) * HI_PER_CORE, :SZ_HI
            ]
            y_lo[c, k * LO_PER_CORE : (k + 1) * LO_PER_CORE, :] = o[
                LO_BASE + c * LO_PER_CORE : LO_BASE + (c + 1) * LO_PER_CORE,
                :SZ_LO,
            ]
    return y_lo, y_hi


def _run_generic(x, j_all, v_all):
    with_scale = not np.all(v_all == 1.0)
    nc = _get_program(("gen", with_scale), lambda: _build_generic(with_scale))
    x_T = np.ascontiguousarray(x.reshape(2, HW).T)  # (65536, 2)
    in_maps = []
    for k in range(N_CORES):
        sl = slice(k * PER_CORE, (k + 1) * PER_CORE)
        m = {"xt": x_T, "idx": np.ascontiguousarray(j_all[sl].reshape(P, CHUNK))}
        if with_scale:
            m["vex"] = np.ascontiguousarray(
                np.repeat(v_all[sl].reshape(P, CHUNK), 2, axis=1).astype(np.float32)
            )
        in_maps.append(m)
    res = _run_spmd(nc, in_maps)
    y_pairs = np.concatenate(
        [np.asarray(res[k]["out"]).reshape(PER_CORE, 2) for k in range(N_CORES)]
    )  # (5120, 2) in (lo, hi) output order, channels last
    y_flat = np.ascontiguousarray(y_pairs.T)
    y_lo = y_flat[:, :N_LO].reshape(2, SZ_LO, SZ_LO).astype(np.float32)
    y_hi = y_flat[:, N_LO:].reshape(2, SZ_HI, SZ_HI).astype(np.float32)
    return y_lo, y_hi


def kernel(x, X_lo, X_hi):
    x = np.ascontiguousarray(np.asarray(x, dtype=np.float32))
    X_lo = np.asarray(X_lo, dtype=np.float32)
    X_hi = np.asarray(X_hi, dtype=np.float32)

    e_lo = _extract_single_nonzero(X_lo)
    e_hi = _extract_single_nonzero(X_hi)

    if e_lo is not None and e_hi is not None:
        (j_lo, v_lo), (j_hi, v_hi) = e_lo, e_hi
        if np.all(v_lo == 1.0) and np.all(v_hi == 1.0):
            sep_hi = _separable(j_hi, SZ_HI)
            sep_lo = _separable(j_lo, SZ_LO)
            if sep_hi is not None and sep_lo is not None:
                ry_hi, cx_hi = sep_hi
                ry_lo, cx_lo = sep_lo
                n_segs = len(_segments(cx_hi)) + len(_segments(cx_lo))
                if n_segs <= 24:
                    return _run_separable(x, ry_hi, cx_hi, ry_lo, cx_lo)
        j_all = np.concatenate([j_lo, j_hi])
        v_all = np.concatenate([v_lo, v_hi])
        return _run_generic(x, j_all, v_all)

    # rows with several nonzeros: decompose into one-hot layers and sum
    lay_lo = _sparse_layers(X_lo)
    lay_hi = _sparse_layers(X_hi)
    L = max(len(lay_lo), len(lay_hi))
    zlo = (np.zeros(N_LO, np.int32), np.zeros(N_LO, np.float32))
    zhi = (np.zeros(N_HI, np.int32), np.zeros(N_HI, np.float32))
    lay_lo += [zlo] * (L - len(lay_lo))
    lay_hi += [zhi] * (L - len(lay_hi))
    acc_lo = np.zeros((2, SZ_LO, SZ_LO), np.float64)
    acc_hi = np.zeros((2, SZ_HI, SZ_HI), np.float64)
    for (jl, vl), (jh, vh) in zip(lay_lo, lay_hi):
        y_lo, y_hi = _run_generic(
            x, np.concatenate([jl, jh]), np.concatenate([vl, vh])
        )
        acc_lo += y_lo
        acc_hi += y_hi
    return acc_lo.astype(np.float32), acc_hi.astype(np.float32)


# revision 16
# speedup vs baseline: 1.0441x; 1.0441x over previous
"""Trainium2 Bass kernel for nn_DownsampleMultiscale (embedding_lookup).

reference:
    xf = x.reshape(2, H*W)                      # x: (2, 256, 256) f32
    y_lo = (xf @ X_lo.T).reshape(2, 32, 32)     # X_lo: (1024, 65536) one-hot rows
    y_hi = (xf @ X_hi.T).reshape(2, 64, 64)     # X_hi: (4096, 65536) one-hot rows

Each row of X_lo / X_hi is one-hot, so each output pixel is a single lookup
into the image: y[c, i] = v_i * x[c, j_i], where j_i is the row's nonzero
column and v_i its value (1.0 here). The host compresses the one-hot
matrices to (j, v) pairs; the device does the lookups from the full image.

Sharding (per the hint): data-parallel over output pixels — each of the 8
NeuronCores produces 1/8 of the downsampled grids (8 of 64 y_hi rows + 4 of
32 y_lo rows). x is replicated.

Fast path (the formula matrices are separable: j = ry[u]*W + cx[v]):
  - per core, ONE indirect DMA gathers its 24 needed image rows (2 ch x 12
    rows, 1KB descriptors) using a small runtime row-index input,
  - the column subsampling (cx) is compile-time: 12 strided tensor-copies
    (split across the vector + gpsimd engines) compact the selected
    columns, one plain DMA writes the core's output slice.
Generic path (any X with single-nonzero rows; also scaled/zero rows):
  - per-element gather: 5 indirect DMAs of 128 single-pixel descriptors
    (both channels per descriptor), optional scale multiply.
Matrices with multi-nonzero rows decompose into L one-hot layers -> L
generic-path launches, summed on the host (L == 1 in practice).
"""

import numpy as np

import concourse.bass as bass
import concourse.mybir as mybir
from concourse.bass_utils import run_bass_kernel_spmd

H, W = 256, 256
HW = H * W                               # 65536
SZ_LO, SZ_HI = 32, 64
N_LO = SZ_LO * SZ_LO                     # 1024
N_HI = SZ_HI * SZ_HI                     # 4096
N_OUT = N_LO + N_HI                      # 5120
N_CORES = 8
PER_CORE = N_OUT // N_CORES              # 640
P = 128
CHUNK = PER_CORE // P                    # 5

HI_PER_CORE = SZ_HI // N_CORES           # 8 y_hi rows per core
LO_PER_CORE = SZ_LO // N_CORES           # 4 y_lo rows per core
N_HI_ROWS = 2 * HI_PER_CORE              # 16 gathered (ch, row) pairs
N_LO_ROWS = 2 * LO_PER_CORE              # 8
LO_BASE = 32                             # lo partition base (32-aligned)
NROWS = LO_BASE + N_LO_ROWS              # 40 gather partitions

F32 = mybir.dt.float32
I32 = mybir.dt.int32

_PROGRAM_CACHE: dict = {}


def _new_bass():
    """Bass() without the init all-engine barrier / NRT pseudo-barrier /
    const-AP memsets / monotonic sems — none of which this kernel needs."""
    orig_pb = bass.Bass._nrt_pseudo_barrier
    orig_aeb = bass.Bass.all_engine_barrier
    bass.Bass._nrt_pseudo_barrier = lambda self: None
    bass.Bass.all_engine_barrier = lambda self, *, sem_only=False: None
    try:
        nc = bass.Bass(monotonic_sem_count=0)
    finally:
        bass.Bass._nrt_pseudo_barrier = orig_pb
        bass.Bass.all_engine_barrier = orig_aeb
    bb = nc.main_func.blocks[0]
    for inst in [i for i in bb.instructions if type(i).__name__ == "InstMemset"]:
        bb.instructions.remove(inst)
    return nc


def _segments(cx):
    """Split a column-index list into maximal dominant-stride segments.

    Returns [(dst_pos, src_start, stride, count)] covering cx in order."""
    from collections import Counter

    cx = list(map(int, cx))
    if len(cx) == 1:
        return [(0, cx[0], 1, 1)]
    d = [b - a for a, b in zip(cx, cx[1:])]
    s_dom = Counter(d).most_common(1)[0][0]
    if s_dom < 1:
        s_dom = 1
    segs = []
    start = 0
    for t in range(1, len(cx)):
        if d[t - 1] != s_dom:
            segs.append((start, cx[start], s_dom, t - start))
            start = t
    segs.append((start, cx[start], s_dom, len(cx) - start))
    return segs


def _pair_jobs(p0, p1, segs):
    """Pair segments with identical (stride, count) into merged copy jobs.

    Job: (p0, p1, dpos, src0, stride, cnt, dgap, sgap); dgap == 0 -> single
    segment, dgap > 0 -> two segments (second at dpos+dgap / src0+sgap)."""
    from collections import defaultdict

    groups = defaultdict(list)
    for d, s, st, c in segs:
        groups[(st, c)].append((d, s))
    jobs = []
    for (st, c), g in groups.items():
        g.sort()
        while g:
            d1, s1 = g.pop(0)
            if g and g[0][0] > d1 and g[0][1] > s1:
                d2, s2 = g.pop(0)
                jobs.append((p0, p1, d1, s1, st, c, d2 - d1, s2 - s1))
            else:
                jobs.append((p0, p1, d1, s1, st, c, 0, 0))
    jobs.sort(key=lambda j: -(j[5] * (2 if j[6] else 1)))
    return jobs


def _build_separable(cx_hi, cx_lo):
    """Row-gather + compile-time column-select program (shared by all cores)."""
    nc = _new_bass()
    xt = nc.dram_tensor("xt", [2 * H, W], F32, kind="ExternalInput")
    ridx = nc.dram_tensor("ridx", [NROWS, 1], I32, kind="ExternalInput")
    out = nc.dram_tensor("out", [NROWS, SZ_HI], F32, kind="ExternalOutput")
    with (
        nc.sbuf_tensor([NROWS, 1], I32) as ridx_t,
        nc.sbuf_tensor([NROWS, W], F32) as rows_t,
        nc.sbuf_tensor([NROWS, SZ_HI], F32) as y_t,
        nc.semaphore() as s_r,
        nc.semaphore() as s_g,
        nc.semaphore() as s_ms,
        nc.semaphore() as s_hi,
        nc.semaphore() as s_lo,
        nc.semaphore() as s_out,
    ):
        nc.sync.dma_start(out=ridx_t[:], in_=ridx[:]).then_inc(s_r, 16)
        # zero the pad regions the copies don't write (off the critical path:
        # runs in the shadow of the ridx DMA receipt)
        nc.gpsimd.memset(y_t[:], 0.0).then_inc(s_ms, 1)
        nc.gpsimd.wait_ge(s_r, 16)
        nc.gpsimd.indirect_dma_start(
            out=rows_t[:],
            out_offset=None,
            in_=xt[:],
            in_offset=bass.IndirectOffsetOnAxis(ap=ridx_t[:, 0:1], axis=0),
        ).then_inc(s_g, 16)
        hi_jobs = _pair_jobs(0, N_HI_ROWS, _segments(cx_hi))
        lo_jobs = _pair_jobs(LO_BASE, LO_BASE + N_LO_ROWS, _segments(cx_lo))
        engines = [nc.vector, nc.gpsimd]
        # gate each engine's copies once: after the memset (gpsimd ops may
        # run concurrently across its DSP cores, so program order is not
        # enough) and after the gathered rows have landed
        for eng in engines:
            eng.wait_ge(s_ms, 1)
            eng.wait_ge(s_g, 16)
        # hi copies first on both engines so the hi output DMA launches while
        # the lo copies are still running
        for jobs, sem in ((hi_jobs, s_hi), (lo_jobs, s_lo)):
            for qi, (p0, p1, dpos, src0, stride, cnt, dgap, sgap) in enumerate(
                jobs
            ):
                eng = engines[qi % len(engines)]
                src = rows_t[p0:p1, src0 : src0 + stride * (cnt - 1) + 1 : stride]
                dst = y_t[p0:p1, dpos : dpos + cnt]
                if dgap:
                    # two segments with identical (stride, count): one copy
                    # with an extra outer AP dim covering both
                    src = bass.AP(
                        src.tensor,
                        src.offset,
                        [list(src.ap[0]), [sgap, 2], list(src.ap[1])],
                    )
                    dst = bass.AP(
                        dst.tensor,
                        dst.offset,
                        [list(dst.ap[0]), [dgap, 2], list(dst.ap[1])],
                    )
                eng.tensor_copy(out=dst, in_=src).then_inc(sem, 1)
        nc.sync.wait_ge(s_hi, len(hi_jobs))
        nc.sync.wait_ge(s_lo, len(lo_jobs))
        nc.sync.dma_start(out=out[:], in_=y_t[:]).then_inc(s_out, 16)
    return nc


def _build_generic(with_scale):
    """Per-element gather: out (p, 2m+c) <- v * x_T[idx[p, m], c]."""
    nc = _new_bass()
    xt = nc.dram_tensor("xt", [HW, 2], F32, kind="ExternalInput")
    idx = nc.dram_tensor("idx", [P, CHUNK], I32, kind="ExternalInput")
    if with_scale:
        vex = nc.dram_tensor("vex", [P, 2 * CHUNK], F32, kind="ExternalInput")
    out = nc.dram_tensor("out", [P, 2 * CHUNK], F32, kind="ExternalOutput")
    with (
        nc.sbuf_tensor([P, CHUNK], I32) as idx_t,
        nc.sbuf_tensor([P, 2 * CHUNK], F32) as g_t,
        nc.sbuf_tensor([P, 2 * CHUNK], F32) as v_t,
        nc.sbuf_tensor([P, 2 * CHUNK], F32) as y_t,
        nc.semaphore() as s_in,
        nc.semaphore() as s_g,
        nc.semaphore() as s_mul,
        nc.semaphore() as s_out,
    ):
        nc.sync.dma_start(out=idx_t[:], in_=idx[:]).then_inc(s_in, 16)
        if with_scale:
            nc.sync.dma_start(out=v_t[:], in_=vex[:]).then_inc(s_in, 16)
        nc.gpsimd.wait_ge(s_in, 32 if with_scale else 16)
        for m in range(CHUNK):
            nc.gpsimd.indirect_dma_start(
                out=g_t[:, 2 * m : 2 * m + 2],
                out_offset=None,
                in_=xt[:],
                in_offset=bass.IndirectOffsetOnAxis(ap=idx_t[:, m : m + 1], axis=0),
            ).then_inc(s_g, 16)
        if with_scale:
            nc.vector.wait_ge(s_g, 16 * CHUNK)
            nc.vector.tensor_mul(out=y_t[:], in0=g_t[:], in1=v_t[:]).then_inc(
                s_mul, 1
            )
            nc.sync.wait_ge(s_mul, 1)
            nc.sync.dma_start(out=out[:], in_=y_t[:]).then_inc(s_out, 16)
        else:
            nc.sync.wait_ge(s_g, 16 * CHUNK)
            nc.sync.dma_start(out=out[:], in_=g_t[:]).then_inc(s_out, 16)
    return nc


def _get_program(key, builder):
    if key not in _PROGRAM_CACHE:
        _PROGRAM_CACHE[key] = builder()
    return _PROGRAM_CACHE[key]


def _run_spmd(nc, in_maps, attempts=3):
    """run_bass_kernel_spmd with retries for transient device errors."""
    for att in range(attempts):
        try:
            return run_bass_kernel_spmd(
                nc, in_maps, core_ids=list(range(N_CORES))
            ).results
        except Exception:
            if att == attempts - 1:
                raise
            import time

            time.sleep(2.0 * (att + 1))


def _extract_single_nonzero(X):
    """(col, val) per row for a matrix with at most one nonzero per row.

    Returns None if some row has more than one nonzero."""
    rows, cols = X.shape
    pos = np.flatnonzero(X)
    r = pos // cols
    if pos.size and np.any(r[1:] == r[:-1]):
        return None  # some row has several nonzeros
    j = np.zeros(rows, np.int32)
    v = np.zeros(rows, np.float32)
    j[r] = (pos % cols).astype(np.int32)
    v[r] = X.ravel()[pos].astype(np.float32)
    return j, v


def _sparse_layers(X):
    """Decompose X into layers of (j, v) with one nonzero per row each."""
    rows, cols = X.shape
    pos = np.flatnonzero(X)
    r = pos // cols
    layers = []
    k = 0
    while pos.size:
        first = np.ones(pos.size, bool)
        first[1:] = r[1:] != r[:-1]
        sel = pos[first]
        j = np.zeros(rows, np.int32)
        v = np.zeros(rows, np.float32)
        rr = sel // cols
        j[rr] = (sel % cols).astype(np.int32)
        v[rr] = X.ravel()[sel].astype(np.float32)
        layers.append((j, v))
        pos, r = pos[~first], r[~first]
        k += 1
        if k > 64:  # pathological density: bail (cannot happen for one-hot)
            raise ValueError("X matrices are too dense for the gather kernel")
    return layers


def _separable(j, sz):
    Jm = j.reshape(sz, sz)
    ry = Jm[:, 0] // W
    cx = Jm[0, :] % W
    if np.array_equal(Jm, ry[:, None] * W + cx[None, :]):
        return ry.astype(np.int64), cx.astype(np.int64)
    return None


def _run_separable(x, ry_hi, cx_hi, ry_lo, cx_lo):
    nc = _get_program(
        ("sep", tuple(cx_hi.tolist()), tuple(cx_lo.tolist())),
        lambda: _build_separable(cx_hi, cx_lo),
    )
    xt = np.ascontiguousarray(x.reshape(2 * H, W))
    in_maps = []
    for k in range(N_CORES):
        rh = ry_hi[k * HI_PER_CORE : (k + 1) * HI_PER_CORE]
        rl = ry_lo[k * LO_PER_CORE : (k + 1) * LO_PER_CORE]
        ridx = np.zeros(NROWS, np.int64)
        ridx[0:HI_PER_CORE] = rh
        ridx[HI_PER_CORE : 2 * HI_PER_CORE] = H + rh
        ridx[LO_BASE : LO_BASE + LO_PER_CORE] = rl
        ridx[LO_BASE + LO_PER_CORE : LO_BASE + 2 * LO_PER_CORE] = H + rl
        in_maps.append(
            {"xt": xt, "ridx": ridx.astype(np.int32).reshape(NROWS, 1)}
        )
    res = _run_spmd(nc, in_maps)
    y_hi = np.empty((2, SZ_HI, SZ_HI), np.float32)
    y_lo = np.empty((2, SZ_LO, SZ_LO), np.float32)
    for k in range(N_CORES):
        o = np.asarray(res[k]["out"])  # (40, 64)
        for c in range(2):
            y_hi[c, k * HI_PER_CORE : (k + 1) * HI_PER_CORE, :] = o[
                c * HI_PER_CORE : (c + 1) * HI_PER_CORE, :SZ_HI
            ]
            y_lo[c, k * LO_PER_CORE : (k + 1) * LO_PER_CORE, :] = o[
                LO_BASE + c * LO_PER_CORE : LO_BASE + (c + 1) * LO_PER_CORE,
                :SZ_LO,
            ]
    return y_lo, y_hi


def _run_generic(x, j_all, v_all):
    with_scale = not np.all(v_all == 1.0)
    nc = _get_program(("gen", with_scale), lambda: _build_generic(with_scale))
    x_T = np.ascontiguousarray(x.reshape(2, HW).T)  # (65536, 2)
    in_maps = []
    for k in range(N_CORES):
        sl = slice(k * PER_CORE, (k + 1) * PER_CORE)
        m = {"xt": x_T, "idx": np.ascontiguousarray(j_all[sl].reshape(P, CHUNK))}
        if with_scale:
            m["vex"] = np.ascontiguousarray(
                np.repeat(v_all[sl].reshape(P, CHUNK), 2, axis=1).astype(np.float32)
            )
        in_maps.append(m)
    res = _run_spmd(nc, in_maps)
    y_pairs = np.concatenate(
        [np.asarray(res[k]["out"]).reshape(PER_CORE, 2) for k in range(N_CORES)]
    )  # (5120, 2) in (lo, hi) output order, channels last
    y_flat = np.ascontiguousarray(y_pairs.T)
    y_lo = y_flat[:, :N_LO].reshape(2, SZ_LO, SZ_LO).astype(np.float32)
    y_hi = y_flat[:, N_LO:].reshape(2, SZ_HI, SZ_HI).astype(np.float32)
    return y_lo, y_hi


def kernel(x, X_lo, X_hi):
    x = np.ascontiguousarray(np.asarray(x, dtype=np.float32))
    X_lo = np.asarray(X_lo, dtype=np.float32)
    X_hi = np.asarray(X_hi, dtype=np.float32)

    e_lo = _extract_single_nonzero(X_lo)
    e_hi = _extract_single_nonzero(X_hi)

    if e_lo is not None and e_hi is not None:
        (j_lo, v_lo), (j_hi, v_hi) = e_lo, e_hi
        if np.all(v_lo == 1.0) and np.all(v_hi == 1.0):
            sep_hi = _separable(j_hi, SZ_HI)
            sep_lo = _separable(j_lo, SZ_LO)
            if sep_hi is not None and sep_lo is not None:
                ry_hi, cx_hi = sep_hi
                ry_lo, cx_lo = sep_lo
                n_segs = len(_segments(cx_hi)) + len(_segments(cx_lo))
                if n_segs <= 24:
                    return _run_separable(x, ry_hi, cx_hi, ry_lo, cx_lo)
        j_all = np.concatenate([j_lo, j_hi])
        v_all = np.concatenate([v_lo, v_hi])
        return _run_generic(x, j_all, v_all)

    # rows with several nonzeros: decompose into one-hot layers and sum
    lay_lo = _sparse_layers(X_lo)
    lay_hi = _sparse_layers(X_hi)
    L = max(len(lay_lo), len(lay_hi))
    zlo = (np.zeros(N_LO, np.int32), np.zeros(N_LO, np.float32))
    zhi = (np.zeros(N_HI, np.int32), np.zeros(N_HI, np.float32))
    lay_lo += [zlo] * (L - len(lay_lo))
    lay_hi += [zhi] * (L - len(lay_hi))
    acc_lo = np.zeros((2, SZ_LO, SZ_LO), np.float64)
    acc_hi = np.zeros((2, SZ_HI, SZ_HI), np.float64)
    for (jl, vl), (jh, vh) in zip(lay_lo, lay_hi):
        y_lo, y_hi = _run_generic(
            x, np.concatenate([jl, jh]), np.concatenate([vl, vh])
        )
        acc_lo += y_lo
        acc_hi += y_hi
    return acc_lo.astype(np.float32), acc_hi.astype(np.float32)


# revision 17
# speedup vs baseline: 1.1999x; 1.1493x over previous
"""Trainium2 Bass kernel for nn_DownsampleMultiscale (embedding_lookup).

reference:
    xf = x.reshape(2, H*W)                      # x: (2, 256, 256) f32
    y_lo = (xf @ X_lo.T).reshape(2, 32, 32)     # X_lo: (1024, 65536) one-hot rows
    y_hi = (xf @ X_hi.T).reshape(2, 64, 64)     # X_hi: (4096, 65536) one-hot rows

Each row of X_lo / X_hi is one-hot, so each output pixel is a single lookup
into the image: y[c, i] = v_i * x[c, j_i], where j_i is the row's nonzero
column and v_i its value (1.0 here). The host compresses the one-hot
matrices to (j, v) pairs; the device does the lookups from the full image.

Sharding (per the hint): data-parallel over output pixels — each of the 8
NeuronCores produces 1/8 of the downsampled grids (8 of 64 y_hi rows + 4 of
32 y_lo rows). x is replicated.

Fast path (the formula matrices are separable: j = ry[u]*W + cx[v]):
  - per core, ONE indirect DMA gathers its 24 needed image rows (2 ch x 12
    rows, 1KB descriptors) using a small runtime row-index input,
  - the column subsampling (cx) is compile-time: 12 strided tensor-copies
    (split across the vector + gpsimd engines) compact the selected
    columns, one plain DMA writes the core's output slice.
Generic path (any X with single-nonzero rows; also scaled/zero rows):
  - per-element gather: 5 indirect DMAs of 128 single-pixel descriptors
    (both channels per descriptor), optional scale multiply.
Matrices with multi-nonzero rows decompose into L one-hot layers -> L
generic-path launches, summed on the host (L == 1 in practice).
"""

import numpy as np

import concourse.bass as bass
import concourse.mybir as mybir
from concourse.bass_utils import run_bass_kernel_spmd

H, W = 256, 256
HW = H * W                               # 65536
SZ_LO, SZ_HI = 32, 64
N_LO = SZ_LO * SZ_LO                     # 1024
N_HI = SZ_HI * SZ_HI                     # 4096
N_OUT = N_LO + N_HI                      # 5120
N_CORES = 8
PER_CORE = N_OUT // N_CORES              # 640
P = 128
CHUNK = PER_CORE // P                    # 5

HI_PER_CORE = SZ_HI // N_CORES           # 8 y_hi rows per core
LO_PER_CORE = SZ_LO // N_CORES           # 4 y_lo rows per core
N_HI_ROWS = 2 * HI_PER_CORE              # 16 gathered (ch, row) pairs
N_LO_ROWS = 2 * LO_PER_CORE              # 8
LO_BASE = 32                             # lo partition base (32-aligned)
NROWS = LO_BASE + N_LO_ROWS              # 40 gather partitions

F32 = mybir.dt.float32
I32 = mybir.dt.int32

_PROGRAM_CACHE: dict = {}


def _new_bass():
    """Bass() without the init all-engine barrier / NRT pseudo-barrier /
    const-AP memsets / monotonic sems — none of which this kernel needs."""
    orig_pb = bass.Bass._nrt_pseudo_barrier
    orig_aeb = bass.Bass.all_engine_barrier
    bass.Bass._nrt_pseudo_barrier = lambda self: None
    bass.Bass.all_engine_barrier = lambda self, *, sem_only=False: None
    try:
        nc = bass.Bass(monotonic_sem_count=0)
    finally:
        bass.Bass._nrt_pseudo_barrier = orig_pb
        bass.Bass.all_engine_barrier = orig_aeb
    bb = nc.main_func.blocks[0]
    for inst in [i for i in bb.instructions if type(i).__name__ == "InstMemset"]:
        bb.instructions.remove(inst)
    return nc


def _segments(cx):
    """Split a column-index list into maximal dominant-stride segments.

    Returns [(dst_pos, src_start, stride, count)] covering cx in order."""
    from collections import Counter

    cx = list(map(int, cx))
    if len(cx) == 1:
        return [(0, cx[0], 1, 1)]
    d = [b - a for a, b in zip(cx, cx[1:])]
    s_dom = Counter(d).most_common(1)[0][0]
    if s_dom < 1:
        s_dom = 1
    segs = []
    start = 0
    for t in range(1, len(cx)):
        if d[t - 1] != s_dom:
            segs.append((start, cx[start], s_dom, t - start))
            start = t
    segs.append((start, cx[start], s_dom, len(cx) - start))
    return segs


def _pair_jobs(p0, p1, segs):
    """Pair segments with identical (stride, count) into merged copy jobs.

    Job: (p0, p1, dpos, src0, stride, cnt, dgap, sgap); dgap == 0 -> single
    segment, dgap > 0 -> two segments (second at dpos+dgap / src0+sgap)."""
    from collections import defaultdict

    groups = defaultdict(list)
    for d, s, st, c in segs:
        groups[(st, c)].append((d, s))
    jobs = []
    for (st, c), g in groups.items():
        g.sort()
        while g:
            d1, s1 = g.pop(0)
            if g and g[0][0] > d1 and g[0][1] > s1:
                d2, s2 = g.pop(0)
                jobs.append((p0, p1, d1, s1, st, c, d2 - d1, s2 - s1))
            else:
                jobs.append((p0, p1, d1, s1, st, c, 0, 0))
    jobs.sort(key=lambda j: -(j[5] * (2 if j[6] else 1)))
    return jobs


def _build_separable(cx_hi, cx_lo):
    """Row-gather + compile-time column-select program (shared by all cores)."""
    nc = _new_bass()
    xt = nc.dram_tensor("xt", [2 * H, W], F32, kind="ExternalInput")
    ridx = nc.dram_tensor("ridx", [NROWS, 1], I32, kind="ExternalInput")
    out = nc.dram_tensor("out", [NROWS, SZ_HI], F32, kind="ExternalOutput")
    with (
        nc.sbuf_tensor([NROWS, 1], I32) as ridx_t,
        nc.sbuf_tensor([NROWS, W], F32) as rows_t,
        nc.sbuf_tensor([NROWS, SZ_HI], F32) as y_t,
        nc.semaphore() as s_r,
        nc.semaphore() as s_g,
        nc.semaphore() as s_ms,
        nc.semaphore() as s_hi,
        nc.semaphore() as s_lo,
        nc.semaphore() as s_out,
    ):
        nc.sync.dma_start(out=ridx_t[:], in_=ridx[:]).then_inc(s_r, 16)
        # zero the pad regions the copies don't write (off the critical path:
        # runs in the shadow of the ridx DMA receipt)
        nc.gpsimd.memset(y_t[:], 0.0).then_inc(s_ms, 1)
        nc.gpsimd.wait_ge(s_r, 16)
        nc.gpsimd.indirect_dma_start(
            out=rows_t[:],
            out_offset=None,
            in_=xt[:],
            in_offset=bass.IndirectOffsetOnAxis(ap=ridx_t[:, 0:1], axis=0),
        ).then_inc(s_g, 16)
        hi_jobs = _pair_jobs(0, N_HI_ROWS, _segments(cx_hi))
        lo_jobs = _pair_jobs(LO_BASE, LO_BASE + N_LO_ROWS, _segments(cx_lo))
        engines = [nc.vector, nc.gpsimd]
        # gate each engine's copies once: after the memset (gpsimd ops may
        # run concurrently across its DSP cores, so program order is not
        # enough) and after the gathered rows have landed
        for eng in engines:
            eng.wait_ge(s_ms, 1)
            eng.wait_ge(s_g, 16)
        # hi copies first on both engines so the hi output DMA launches while
        # the lo copies are still running
        for jobs, sem in ((hi_jobs, s_hi), (lo_jobs, s_lo)):
            for qi, (p0, p1, dpos, src0, stride, cnt, dgap, sgap) in enumerate(
                jobs
            ):
                eng = engines[qi % len(engines)]
                src = rows_t[p0:p1, src0 : src0 + stride * (cnt - 1) + 1 : stride]
                dst = y_t[p0:p1, dpos : dpos + cnt]
                if dgap:
                    # two segments with identical (stride, count): one copy
                    # with an extra outer AP dim covering both
                    src = bass.AP(
                        src.tensor,
                        src.offset,
                        [list(src.ap[0]), [sgap, 2], list(src.ap[1])],
                    )
                    dst = bass.AP(
                        dst.tensor,
                        dst.offset,
                        [list(dst.ap[0]), [dgap, 2], list(dst.ap[1])],
                    )
                eng.tensor_copy(out=dst, in_=src).then_inc(sem, 1)
        # the exit token-chain visits sync early and scalar late: issuing the
        # output DMA from scalar hides its write-receipt wait inside the
        # earlier engines' semaphore-reset epilogue
        nc.scalar.wait_ge(s_hi, len(hi_jobs))
        nc.scalar.wait_ge(s_lo, len(lo_jobs))
        nc.scalar.dma_start(out=out[:], in_=y_t[:]).then_inc(s_out, 16)
    return nc


def _build_generic(with_scale):
    """Per-element gather: out (p, 2m+c) <- v * x_T[idx[p, m], c]."""
    nc = _new_bass()
    xt = nc.dram_tensor("xt", [HW, 2], F32, kind="ExternalInput")
    idx = nc.dram_tensor("idx", [P, CHUNK], I32, kind="ExternalInput")
    if with_scale:
        vex = nc.dram_tensor("vex", [P, 2 * CHUNK], F32, kind="ExternalInput")
    out = nc.dram_tensor("out", [P, 2 * CHUNK], F32, kind="ExternalOutput")
    with (
        nc.sbuf_tensor([P, CHUNK], I32) as idx_t,
        nc.sbuf_tensor([P, 2 * CHUNK], F32) as g_t,
        nc.sbuf_tensor([P, 2 * CHUNK], F32) as v_t,
        nc.sbuf_tensor([P, 2 * CHUNK], F32) as y_t,
        nc.semaphore() as s_in,
        nc.semaphore() as s_g,
        nc.semaphore() as s_mul,
        nc.semaphore() as s_out,
    ):
        nc.sync.dma_start(out=idx_t[:], in_=idx[:]).then_inc(s_in, 16)
        if with_scale:
            nc.sync.dma_start(out=v_t[:], in_=vex[:]).then_inc(s_in, 16)
        nc.gpsimd.wait_ge(s_in, 32 if with_scale else 16)
        for m in range(CHUNK):
            nc.gpsimd.indirect_dma_start(
                out=g_t[:, 2 * m : 2 * m + 2],
                out_offset=None,
                in_=xt[:],
                in_offset=bass.IndirectOffsetOnAxis(ap=idx_t[:, m : m + 1], axis=0),
            ).then_inc(s_g, 16)
        if with_scale:
            nc.vector.wait_ge(s_g, 16 * CHUNK)
            nc.vector.tensor_mul(out=y_t[:], in0=g_t[:], in1=v_t[:]).then_inc(
                s_mul, 1
            )
            nc.sync.wait_ge(s_mul, 1)
            nc.sync.dma_start(out=out[:], in_=y_t[:]).then_inc(s_out, 16)
        else:
            nc.sync.wait_ge(s_g, 16 * CHUNK)
            nc.sync.dma_start(out=out[:], in_=g_t[:]).then_inc(s_out, 16)
    return nc


def _get_program(key, builder):
    if key not in _PROGRAM_CACHE:
        _PROGRAM_CACHE[key] = builder()
    return _PROGRAM_CACHE[key]


def _run_spmd(nc, in_maps, attempts=3):
    """run_bass_kernel_spmd with retries for transient device errors."""
    for att in range(attempts):
        try:
            return run_bass_kernel_spmd(
                nc, in_maps, core_ids=list(range(N_CORES))
            ).results
        except Exception:
            if att == attempts - 1:
                raise
            import time

            time.sleep(2.0 * (att + 1))


def _extract_single_nonzero(X):
    """(col, val) per row for a matrix with at most one nonzero per row.

    Returns None if some row has more than one nonzero."""
    rows, cols = X.shape
    pos = np.flatnonzero(X)
    r = pos // cols
    if pos.size and np.any(r[1:] == r[:-1]):
        return None  # some row has several nonzeros
    j = np.zeros(rows, np.int32)
    v = np.zeros(rows, np.float32)
    j[r] = (pos % cols).astype(np.int32)
    v[r] = X.ravel()[pos].astype(np.float32)
    return j, v


def _sparse_layers(X):
    """Decompose X into layers of (j, v) with one nonzero per row each."""
    rows, cols = X.shape
    pos = np.flatnonzero(X)
    r = pos // cols
    layers = []
    k = 0
    while pos.size:
        first = np.ones(pos.size, bool)
        first[1:] = r[1:] != r[:-1]
        sel = pos[first]
        j = np.zeros(rows, np.int32)
        v = np.zeros(rows, np.float32)
        rr = sel // cols
        j[rr] = (sel % cols).astype(np.int32)
        v[rr] = X.ravel()[sel].astype(np.float32)
        layers.append((j, v))
        pos, r = pos[~first], r[~first]
        k += 1
        if k > 64:  # pathological density: bail (cannot happen for one-hot)
            raise ValueError("X matrices are too dense for the gather kernel")
    return layers


def _separable(j, sz):
    Jm = j.reshape(sz, sz)
    ry = Jm[:, 0] // W
    cx = Jm[0, :] % W
    if np.array_equal(Jm, ry[:, None] * W + cx[None, :]):
        return ry.astype(np.int64), cx.astype(np.int64)
    return None


def _run_separable(x, ry_hi, cx_hi, ry_lo, cx_lo):
    nc = _get_program(
        ("sep", tuple(cx_hi.tolist()), tuple(cx_lo.tolist())),
        lambda: _build_separable(cx_hi, cx_lo),
    )
    xt = np.ascontiguousarray(x.reshape(2 * H, W))
    in_maps = []
    for k in range(N_CORES):
        rh = ry_hi[k * HI_PER_CORE : (k + 1) * HI_PER_CORE]
        rl = ry_lo[k * LO_PER_CORE : (k + 1) * LO_PER_CORE]
        ridx = np.zeros(NROWS, np.int64)
        ridx[0:HI_PER_CORE] = rh
        ridx[HI_PER_CORE : 2 * HI_PER_CORE] = H + rh
        ridx[LO_BASE : LO_BASE + LO_PER_CORE] = rl
        ridx[LO_BASE + LO_PER_CORE : LO_BASE + 2 * LO_PER_CORE] = H + rl
        in_maps.append(
            {"xt": xt, "ridx": ridx.astype(np.int32).reshape(NROWS, 1)}
        )
    res = _run_spmd(nc, in_maps)
    y_hi = np.empty((2, SZ_HI, SZ_HI), np.float32)
    y_lo = np.empty((2, SZ_LO, SZ_LO), np.float32)
    for k in range(N_CORES):
        o = np.asarray(res[k]["out"])  # (40, 64)
        for c in range(2):
            y_hi[c, k * HI_PER_CORE : (k + 1) * HI_PER_CORE, :] = o[
                c * HI_PER_CORE : (c + 1) * HI_PER_CORE, :SZ_HI
            ]
            y_lo[c, k * LO_PER_CORE : (k + 1) * LO_PER_CORE, :] = o[
                LO_BASE + c * LO_PER_CORE : LO_BASE + (c + 1) * LO_PER_CORE,
                :SZ_LO,
            ]
    return y_lo, y_hi


def _run_generic(x, j_all, v_all):
    with_scale = not np.all(v_all == 1.0)
    nc = _get_program(("gen", with_scale), lambda: _build_generic(with_scale))
    x_T = np.ascontiguousarray(x.reshape(2, HW).T)  # (65536, 2)
    in_maps = []
    for k in range(N_CORES):
        sl = slice(k * PER_CORE, (k + 1) * PER_CORE)
        m = {"xt": x_T, "idx": np.ascontiguousarray(j_all[sl].reshape(P, CHUNK))}
        if with_scale:
            m["vex"] = np.ascontiguousarray(
                np.repeat(v_all[sl].reshape(P, CHUNK), 2, axis=1).astype(np.float32)
            )
        in_maps.append(m)
    res = _run_spmd(nc, in_maps)
    y_pairs = np.concatenate(
        [np.asarray(res[k]["out"]).reshape(PER_CORE, 2) for k in range(N_CORES)]
    )  # (5120, 2) in (lo, hi) output order, channels last
    y_flat = np.ascontiguousarray(y_pairs.T)
    y_lo = y_flat[:, :N_LO].reshape(2, SZ_LO, SZ_LO).astype(np.float32)
    y_hi = y_flat[:, N_LO:].reshape(2, SZ_HI, SZ_HI).astype(np.float32)
    return y_lo, y_hi


def kernel(x, X_lo, X_hi):
    x = np.ascontiguousarray(np.asarray(x, dtype=np.float32))
    X_lo = np.asarray(X_lo, dtype=np.float32)
    X_hi = np.asarray(X_hi, dtype=np.float32)

    e_lo = _extract_single_nonzero(X_lo)
    e_hi = _extract_single_nonzero(X_hi)

    if e_lo is not None and e_hi is not None:
        (j_lo, v_lo), (j_hi, v_hi) = e_lo, e_hi
        if np.all(v_lo == 1.0) and np.all(v_hi == 1.0):
            sep_hi = _separable(j_hi, SZ_HI)
            sep_lo = _separable(j_lo, SZ_LO)
            if sep_hi is not None and sep_lo is not None:
                ry_hi, cx_hi = sep_hi
                ry_lo, cx_lo = sep_lo
                n_segs = len(_segments(cx_hi)) + len(_segments(cx_lo))
                if n_segs <= 24:
                    return _run_separable(x, ry_hi, cx_hi, ry_lo, cx_lo)
        j_all = np.concatenate([j_lo, j_hi])
        v_all = np.concatenate([v_lo, v_hi])
        return _run_generic(x, j_all, v_all)

    # rows with several nonzeros: decompose into one-hot layers and sum
    lay_lo = _sparse_layers(X_lo)
    lay_hi = _sparse_layers(X_hi)
    L = max(len(lay_lo), len(lay_hi))
    zlo = (np.zeros(N_LO, np.int32), np.zeros(N_LO, np.float32))
    zhi = (np.zeros(N_HI, np.int32), np.zeros(N_HI, np.float32))
    lay_lo += [zlo] * (L - len(lay_lo))
    lay_hi += [zhi] * (L - len(lay_hi))
    acc_lo = np.zeros((2, SZ_LO, SZ_LO), np.float64)
    acc_hi = np.zeros((2, SZ_HI, SZ_HI), np.float64)
    for (jl, vl), (jh, vh) in zip(lay_lo, lay_hi):
        y_lo, y_hi = _run_generic(
            x, np.concatenate([jl, jh]), np.concatenate([vl, vh])
        )
        acc_lo += y_lo
        acc_hi += y_hi
    return acc_lo.astype(np.float32), acc_hi.astype(np.float32)


# revision 18
# speedup vs baseline: 1.2173x; 1.0145x over previous
"""Trainium2 Bass kernel for nn_DownsampleMultiscale (embedding_lookup).

reference:
    xf = x.reshape(2, H*W)                      # x: (2, 256, 256) f32
    y_lo = (xf @ X_lo.T).reshape(2, 32, 32)     # X_lo: (1024, 65536) one-hot rows
    y_hi = (xf @ X_hi.T).reshape(2, 64, 64)     # X_hi: (4096, 65536) one-hot rows

Each row of X_lo / X_hi is one-hot, so each output pixel is a single lookup
into the image: y[c, i] = v_i * x[c, j_i], where j_i is the row's nonzero
column and v_i its value (1.0 here). The host compresses the one-hot
matrices to (j, v) pairs; the device does the lookups from the full image.

Sharding (per the hint): data-parallel over output pixels — each of the 8
NeuronCores produces 1/8 of the downsampled grids (8 of 64 y_hi rows + 4 of
32 y_lo rows). x is replicated.

Fast path (the formula matrices are separable: j = ry[u]*W + cx[v]):
  - per core, ONE indirect DMA gathers its 24 needed image rows (2 ch x 12
    rows, 1KB descriptors) using a small runtime row-index input,
  - the column subsampling (cx) is compile-time: 12 strided tensor-copies
    (split across the vector + gpsimd engines) compact the selected
    columns, one plain DMA writes the core's output slice.
Generic path (any X with single-nonzero rows; also scaled/zero rows):
  - per-element gather: 5 indirect DMAs of 128 single-pixel descriptors
    (both channels per descriptor), optional scale multiply.
Matrices with multi-nonzero rows decompose into L one-hot layers -> L
generic-path launches, summed on the host (L == 1 in practice).
"""

import numpy as np

import concourse.bass as bass
import concourse.mybir as mybir
from concourse.bass_utils import run_bass_kernel_spmd

H, W = 256, 256
HW = H * W                               # 65536
SZ_LO, SZ_HI = 32, 64
N_LO = SZ_LO * SZ_LO                     # 1024
N_HI = SZ_HI * SZ_HI                     # 4096
N_OUT = N_LO + N_HI                      # 5120
N_CORES = 8
PER_CORE = N_OUT // N_CORES              # 640
P = 128
CHUNK = PER_CORE // P                    # 5

HI_PER_CORE = SZ_HI // N_CORES           # 8 y_hi rows per core
LO_PER_CORE = SZ_LO // N_CORES           # 4 y_lo rows per core
N_HI_ROWS = 2 * HI_PER_CORE              # 16 gathered (ch, row) pairs
N_LO_ROWS = 2 * LO_PER_CORE              # 8
LO_BASE = 32                             # lo partition base (32-aligned)
NROWS = LO_BASE + N_LO_ROWS              # 40 gather partitions

F32 = mybir.dt.float32
I32 = mybir.dt.int32

_PROGRAM_CACHE: dict = {}


def _new_bass():
    """Bass() without the init all-engine barrier / NRT pseudo-barrier /
    const-AP memsets / monotonic sems — none of which this kernel needs."""
    orig_pb = bass.Bass._nrt_pseudo_barrier
    orig_aeb = bass.Bass.all_engine_barrier
    bass.Bass._nrt_pseudo_barrier = lambda self: None
    bass.Bass.all_engine_barrier = lambda self, *, sem_only=False: None
    try:
        nc = bass.Bass(monotonic_sem_count=0)
    finally:
        bass.Bass._nrt_pseudo_barrier = orig_pb
        bass.Bass.all_engine_barrier = orig_aeb
    bb = nc.main_func.blocks[0]
    for inst in [i for i in bb.instructions if type(i).__name__ == "InstMemset"]:
        bb.instructions.remove(inst)
    return nc


def _segments(cx):
    """Split a column-index list into maximal dominant-stride segments.

    Returns [(dst_pos, src_start, stride, count)] covering cx in order."""
    from collections import Counter

    cx = list(map(int, cx))
    if len(cx) == 1:
        return [(0, cx[0], 1, 1)]
    d = [b - a for a, b in zip(cx, cx[1:])]
    s_dom = Counter(d).most_common(1)[0][0]
    if s_dom < 1:
        s_dom = 1
    segs = []
    start = 0
    for t in range(1, len(cx)):
        if d[t - 1] != s_dom:
            segs.append((start, cx[start], s_dom, t - start))
            start = t
    segs.append((start, cx[start], s_dom, len(cx) - start))
    return segs


def _pair_jobs(p0, p1, segs):
    """Pair segments with identical (stride, count) into merged copy jobs.

    Job: (p0, p1, dpos, src0, stride, cnt, dgap, sgap); dgap == 0 -> single
    segment, dgap > 0 -> two segments (second at dpos+dgap / src0+sgap)."""
    from collections import defaultdict

    groups = defaultdict(list)
    for d, s, st, c in segs:
        groups[(st, c)].append((d, s))
    jobs = []
    for (st, c), g in groups.items():
        g.sort()
        while g:
            d1, s1 = g.pop(0)
            if g and g[0][0] > d1 and g[0][1] > s1:
                d2, s2 = g.pop(0)
                jobs.append((p0, p1, d1, s1, st, c, d2 - d1, s2 - s1))
            else:
                jobs.append((p0, p1, d1, s1, st, c, 0, 0))
    jobs.sort(key=lambda j: -(j[5] * (2 if j[6] else 1)))
    return jobs


def _build_separable(cx_hi, cx_lo):
    """Row-gather + compile-time column-select program (shared by all cores)."""
    nc = _new_bass()
    xt = nc.dram_tensor("xt", [2 * H, W], F32, kind="ExternalInput")
    ridx = nc.dram_tensor("ridx", [NROWS, 1], I32, kind="ExternalInput")
    out = nc.dram_tensor("out", [NROWS, SZ_HI], F32, kind="ExternalOutput")
    with (
        nc.sbuf_tensor([NROWS, 1], I32) as ridx_t,
        nc.sbuf_tensor([NROWS, W], F32) as rows_t,
        nc.sbuf_tensor([NROWS, SZ_HI], F32) as y_t,
        nc.semaphore() as s_r,
        nc.semaphore() as s_g,
        nc.semaphore() as s_ms,
        nc.semaphore() as s_hi,
        nc.semaphore() as s_lo,
        nc.semaphore() as s_out,
    ):
        nc.sync.dma_start(out=ridx_t[:], in_=ridx[:]).then_inc(s_r, 16)
        # zero the pad regions the copies don't write (off the critical path:
        # runs in the shadow of the ridx DMA receipt)
        nc.gpsimd.memset(y_t[:], 0.0).then_inc(s_ms, 1)
        nc.gpsimd.wait_ge(s_r, 16)
        nc.gpsimd.indirect_dma_start(
            out=rows_t[:],
            out_offset=None,
            in_=xt[:],
            in_offset=bass.IndirectOffsetOnAxis(ap=ridx_t[:, 0:1], axis=0),
        ).then_inc(s_g, 16)
        hi_jobs = _pair_jobs(0, N_HI_ROWS, _segments(cx_hi))
        lo_jobs = _pair_jobs(LO_BASE, LO_BASE + N_LO_ROWS, _segments(cx_lo))
        engines = [nc.vector, nc.gpsimd]
        # gate each engine's copies once: after the memset (gpsimd ops may
        # run concurrently across its DSP cores, so program order is not
        # enough) and after the gathered rows have landed
        for eng in engines:
            eng.wait_ge(s_ms, 1)
            eng.wait_ge(s_g, 16)
        # hi copies first on both engines so the hi output DMA launches while
        # the lo copies are still running
        for jobs, sem in ((hi_jobs, s_hi), (lo_jobs, s_lo)):
            for qi, (p0, p1, dpos, src0, stride, cnt, dgap, sgap) in enumerate(
                jobs
            ):
                eng = engines[qi % len(engines)]
                src = rows_t[p0:p1, src0 : src0 + stride * (cnt - 1) + 1 : stride]
                dst = y_t[p0:p1, dpos : dpos + cnt]
                if dgap:
                    # two segments with identical (stride, count): one copy
                    # with an extra outer AP dim covering both
                    src = bass.AP(
                        src.tensor,
                        src.offset,
                        [list(src.ap[0]), [sgap, 2], list(src.ap[1])],
                    )
                    dst = bass.AP(
                        dst.tensor,
                        dst.offset,
                        [list(dst.ap[0]), [dgap, 2], list(dst.ap[1])],
                    )
                eng.tensor_copy(out=dst, in_=src).then_inc(sem, 1)
        nc.sync.wait_ge(s_hi, len(hi_jobs))
        nc.sync.wait_ge(s_lo, len(lo_jobs))
        nc.sync.dma_start(out=out[:], in_=y_t[:]).then_inc(s_out, 16)
    return nc


def _build_generic(with_scale):
    """Per-element gather: out (p, 2m+c) <- v * x_T[idx[p, m], c]."""
    nc = _new_bass()
    xt = nc.dram_tensor("xt", [HW, 2], F32, kind="ExternalInput")
    idx = nc.dram_tensor("idx", [P, CHUNK], I32, kind="ExternalInput")
    if with_scale:
        vex = nc.dram_tensor("vex", [P, 2 * CHUNK], F32, kind="ExternalInput")
    out = nc.dram_tensor("out", [P, 2 * CHUNK], F32, kind="ExternalOutput")
    with (
        nc.sbuf_tensor([P, CHUNK], I32) as idx_t,
        nc.sbuf_tensor([P, 2 * CHUNK], F32) as g_t,
        nc.sbuf_tensor([P, 2 * CHUNK], F32) as v_t,
        nc.sbuf_tensor([P, 2 * CHUNK], F32) as y_t,
        nc.semaphore() as s_in,
        nc.semaphore() as s_g,
        nc.semaphore() as s_mul,
        nc.semaphore() as s_out,
    ):
        nc.sync.dma_start(out=idx_t[:], in_=idx[:]).then_inc(s_in, 16)
        if with_scale:
            nc.sync.dma_start(out=v_t[:], in_=vex[:]).then_inc(s_in, 16)
        nc.gpsimd.wait_ge(s_in, 32 if with_scale else 16)
        for m in range(CHUNK):
            nc.gpsimd.indirect_dma_start(
                out=g_t[:, 2 * m : 2 * m + 2],
                out_offset=None,
                in_=xt[:],
                in_offset=bass.IndirectOffsetOnAxis(ap=idx_t[:, m : m + 1], axis=0),
            ).then_inc(s_g, 16)
        if with_scale:
            nc.vector.wait_ge(s_g, 16 * CHUNK)
            nc.vector.tensor_mul(out=y_t[:], in0=g_t[:], in1=v_t[:]).then_inc(
                s_mul, 1
            )
            nc.sync.wait_ge(s_mul, 1)
            nc.sync.dma_start(out=out[:], in_=y_t[:]).then_inc(s_out, 16)
        else:
            nc.sync.wait_ge(s_g, 16 * CHUNK)
            nc.sync.dma_start(out=out[:], in_=g_t[:]).then_inc(s_out, 16)
    return nc


def _get_program(key, builder):
    if key not in _PROGRAM_CACHE:
        _PROGRAM_CACHE[key] = builder()
    return _PROGRAM_CACHE[key]


def _run_spmd(nc, in_maps, attempts=3):
    """run_bass_kernel_spmd with retries for transient device errors."""
    for att in range(attempts):
        try:
            return run_bass_kernel_spmd(
                nc, in_maps, core_ids=list(range(N_CORES))
            ).results
        except Exception:
            if att == attempts - 1:
                raise
            import time

            time.sleep(2.0 * (att + 1))


def _extract_single_nonzero(X):
    """(col, val) per row for a matrix with at most one nonzero per row.

    Returns None if some row has more than one nonzero."""
    rows, cols = X.shape
    pos = np.flatnonzero(X)
    r = pos // cols
    if pos.size and np.any(r[1:] == r[:-1]):
        return None  # some row has several nonzeros
    j = np.zeros(rows, np.int32)
    v = np.zeros(rows, np.float32)
    j[r] = (pos % cols).astype(np.int32)
    v[r] = X.ravel()[pos].astype(np.float32)
    return j, v


def _sparse_layers(X):
    """Decompose X into layers of (j, v) with one nonzero per row each."""
    rows, cols = X.shape
    pos = np.flatnonzero(X)
    r = pos // cols
    layers = []
    k = 0
    while pos.size:
        first = np.ones(pos.size, bool)
        first[1:] = r[1:] != r[:-1]
        sel = pos[first]
        j = np.zeros(rows, np.int32)
        v = np.zeros(rows, np.float32)
        rr = sel // cols
        j[rr] = (sel % cols).astype(np.int32)
        v[rr] = X.ravel()[sel].astype(np.float32)
        layers.append((j, v))
        pos, r = pos[~first], r[~first]
        k += 1
        if k > 64:  # pathological density: bail (cannot happen for one-hot)
            raise ValueError("X matrices are too dense for the gather kernel")
    return layers


def _separable(j, sz):
    Jm = j.reshape(sz, sz)
    ry = Jm[:, 0] // W
    cx = Jm[0, :] % W
    if np.array_equal(Jm, ry[:, None] * W + cx[None, :]):
        return ry.astype(np.int64), cx.astype(np.int64)
    return None


def _run_separable(x, ry_hi, cx_hi, ry_lo, cx_lo):
    nc = _get_program(
        ("sep", tuple(cx_hi.tolist()), tuple(cx_lo.tolist())),
        lambda: _build_separable(cx_hi, cx_lo),
    )
    xt = np.ascontiguousarray(x.reshape(2 * H, W))
    in_maps = []
    for k in range(N_CORES):
        rh = ry_hi[k * HI_PER_CORE : (k + 1) * HI_PER_CORE]
        rl = ry_lo[k * LO_PER_CORE : (k + 1) * LO_PER_CORE]
        ridx = np.zeros(NROWS, np.int64)
        ridx[0:HI_PER_CORE] = rh
        ridx[HI_PER_CORE : 2 * HI_PER_CORE] = H + rh
        ridx[LO_BASE : LO_BASE + LO_PER_CORE] = rl
        ridx[LO_BASE + LO_PER_CORE : LO_BASE + 2 * LO_PER_CORE] = H + rl
        in_maps.append(
            {"xt": xt, "ridx": ridx.astype(np.int32).reshape(NROWS, 1)}
        )
    res = _run_spmd(nc, in_maps)
    y_hi = np.empty((2, SZ_HI, SZ_HI), np.float32)
    y_lo = np.empty((2, SZ_LO, SZ_LO), np.float32)
    for k in range(N_CORES):
        o = np.asarray(res[k]["out"])  # (40, 64)
        for c in range(2):
            y_hi[c, k * HI_PER_CORE : (k + 1) * HI_PER_CORE, :] = o[
                c * HI_PER_CORE : (c + 1) * HI_PER_CORE, :SZ_HI
            ]
            y_lo[c, k * LO_PER_CORE : (k + 1) * LO_PER_CORE, :] = o[
                LO_BASE + c * LO_PER_CORE : LO_BASE + (c + 1) * LO_PER_CORE,
                :SZ_LO,
            ]
    return y_lo, y_hi


def _run_generic(x, j_all, v_all):
    with_scale = not np.all(v_all == 1.0)
    nc = _get_program(("gen", with_scale), lambda: _build_generic(with_scale))
    x_T = np.ascontiguousarray(x.reshape(2, HW).T)  # (65536, 2)
    in_maps = []
    for k in range(N_CORES):
        sl = slice(k * PER_CORE, (k + 1) * PER_CORE)
        m = {"xt": x_T, "idx": np.ascontiguousarray(j_all[sl].reshape(P, CHUNK))}
        if with_scale:
            m["vex"] = np.ascontiguousarray(
                np.repeat(v_all[sl].reshape(P, CHUNK), 2, axis=1).astype(np.float32)
            )
        in_maps.append(m)
    res = _run_spmd(nc, in_maps)
    y_pairs = np.concatenate(
        [np.asarray(res[k]["out"]).reshape(PER_CORE, 2) for k in range(N_CORES)]
    )  # (5120, 2) in (lo, hi) output order, channels last
    y_flat = np.ascontiguousarray(y_pairs.T)
    y_lo = y_flat[:, :N_LO].reshape(2, SZ_LO, SZ_LO).astype(np.float32)
    y_hi = y_flat[:, N_LO:].reshape(2, SZ_HI, SZ_HI).astype(np.float32)
    return y_lo, y_hi


def kernel(x, X_lo, X_hi):
    x = np.ascontiguousarray(np.asarray(x, dtype=np.float32))
    X_lo = np.asarray(X_lo, dtype=np.float32)
    X_hi = np.asarray(X_hi, dtype=np.float32)

    e_lo = _extract_single_nonzero(X_lo)
    e_hi = _extract_single_nonzero(X_hi)

    if e_lo is not None and e_hi is not None:
        (j_lo, v_lo), (j_hi, v_hi) = e_lo, e_hi
        if np.all(v_lo == 1.0) and np.all(v_hi == 1.0):
            sep_hi = _separable(j_hi, SZ_HI)
            sep_lo = _separable(j_lo, SZ_LO)
            if sep_hi is not None and sep_lo is not None:
                ry_hi, cx_hi = sep_hi
                ry_lo, cx_lo = sep_lo
                n_segs = len(_segments(cx_hi)) + len(_segments(cx_lo))
                if n_segs <= 24:
                    return _run_separable(x, ry_hi, cx_hi, ry_lo, cx_lo)
        j_all = np.concatenate([j_lo, j_hi])
        v_all = np.concatenate([v_lo, v_hi])
        return _run_generic(x, j_all, v_all)

    # rows with several nonzeros: decompose into one-hot layers and sum
    lay_lo = _sparse_layers(X_lo)
    lay_hi = _sparse_layers(X_hi)
    L = max(len(lay_lo), len(lay_hi))
    zlo = (np.zeros(N_LO, np.int32), np.zeros(N_LO, np.float32))
    zhi = (np.zeros(N_HI, np.int32), np.zeros(N_HI, np.float32))
    lay_lo += [zlo] * (L - len(lay_lo))
    lay_hi += [zhi] * (L - len(lay_hi))
    acc_lo = np.zeros((2, SZ_LO, SZ_LO), np.float64)
    acc_hi = np.zeros((2, SZ_HI, SZ_HI), np.float64)
    for (jl, vl), (jh, vh) in zip(lay_lo, lay_hi):
        y_lo, y_hi = _run_generic(
            x, np.concatenate([jl, jh]), np.concatenate([vl, vh])
        )
        acc_lo += y_lo
        acc_hi += y_hi
    return acc_lo.astype(np.float32), acc_hi.astype(np.float32)
